# revision 4
# baseline (speedup 1.0000x reference)
"""TransE-style GNN message-passing scoring kernel for 8 Trainium2 NeuronCores.

Math: reference computes scores[r,e] = sum_d(ent[src]+rel[rl]-ent[dst])[d].
The sum over d is linear, so scores = S[src] + R[rl] - S[dst] where
S = rowsum(ent_table) [1M], R = rowsum(rel_table) [1000].

Per-core plan (SPMD, core c handles relation row c):
  phase 1: stream 1/8 of ent_table (992 tiles of 128 rows), DVE-reduce each
           [128,128] tile over the free axis -> S-chunk in SBUF [128, 992].
  phase 1b: 32x32 stream-transposes + block-permuting DMA write the chunk to
           DRAM in natural row order; AllGather -> Sg[1,015,808] f32 = S.
           rel rowsums computed locally -> Rg[1024] natural order.
  phase 2: two-level gather: hi = idx>>6 (int16, host-prepped in the SWDGE
           wrapped layout) drives dma_gather of 64-f32 granules from
           Sg viewed [15872, 64] across 4 SWDGE queues; the within-granule
           element is selected on DVE with an iota/is_equal one-hot against
           lo = idx&63 (host-prepped f32), multiply + reduce.
  phase 3: score = sel(src) + sel(rel) - sel(dst); stream-transpose +
           block-permuted DMA writes out[131072] in edge order; zero tail.
Host does only integer index prep (hi/lo split + SWDGE wrap layout) and
tensor sharding/concat; all FP math runs on device.
"""

import numpy as np

N_ENT = 1_000_000
DIM = 128
R_TYPES = 8
E_PER_TYPE = 131_072
SCORE_DIM = 150_000
N_REL = 1_000
N_CORES = 8

FULL_CFG = dict(
    tiles=992,          # ent tiles of 128 rows per core (992*128 = 126,976)
    e_cols=E_PER_TYPE // 128,   # 1024 score slots ([128, e_cols] per core)
    jch=8192,           # indices per dma_gather instruction
    score_dim=SCORE_DIM,
    batch=16,           # stream tiles per DMA batch
    queues=4,
)


def build_nc(cfg):
    import concourse.bass as bass
    import concourse.bacc as bacc
    import concourse.tile as tile
    from concourse import mybir

    f32 = mybir.dt.float32
    i16 = mybir.dt.int16
    AX = mybir.AxisListType
    OP = mybir.AluOpType

    TILES = cfg["tiles"]
    S = cfg["e_cols"]              # score slots (free dim of [128, S])
    JCH = cfg["jch"]
    SCORE = cfg["score_dim"]
    BATCH = cfg["batch"]
    NQ = cfg["queues"]
    ROWS = TILES * 128
    SG_LEN = N_CORES * ROWS        # all-gathered S length (>= N_ENT)
    SG_ROWS = SG_LEN // 64
    NEDGE = 128 * S
    HCOLS = NEDGE // 16            # wrapped-idx columns
    NB = TILES // BATCH
    NG = TILES // 32
    NCH = NEDGE // JCH             # gather chunks per stream
    SC = JCH // 128                # score slots per chunk
    CHW = JCH // 16                # hi columns per chunk
    NG2 = S // 32
    assert TILES % BATCH == 0 and TILES % 32 == 0 and NEDGE % JCH == 0
    assert JCH % 128 == 0 and S % 32 == 0
    PAD = SCORE - NEDGE
    assert PAD % 16 == 0

    nc = bacc.Bacc(None, num_devices=N_CORES, num_swdge_queues=NQ)
    ent = nc.dram_tensor("ent_shard", [ROWS, DIM], f32, kind="ExternalInput")
    rel = nc.dram_tensor("rel_table", [N_REL, DIM], f32, kind="ExternalInput")
    his = {}
    los = {}
    for st in ("src", "dst", "rel"):
        his[st] = nc.dram_tensor(f"hi_{st}", [128, HCOLS], i16, kind="ExternalInput")
        los[st] = nc.dram_tensor(f"lo_{st}", [128, S], f32, kind="ExternalInput")
    out = nc.dram_tensor("out", [SCORE], f32, kind="ExternalOutput")
    iota = nc.inline_tensor(
        np.tile(np.arange(64, dtype=np.float32), (128, 1)), name="iota64")

    with tile.TileContext(nc) as tc:
        with tc.tile_pool(name="stream", bufs=3) as p_st, \
             tc.tile_pool(name="persist", bufs=1) as p_p, \
             tc.tile_pool(name="gout", bufs=3) as p_go, \
             tc.tile_pool(name="mask", bufs=2) as p_mk, \
             tc.tile_pool(name="dram", bufs=1, space="DRAM") as p_d:

            ag_in = p_d.tile([ROWS], f32)
            sg = p_d.tile([SG_LEN], f32, addr_space="Shared")
            rg = p_d.tile([1024], f32)

            # ---- index metadata loads (early) ----
            hi_sb = {}
            lo_sb = {}
            for st in ("src", "dst", "rel"):
                hi_sb[st] = p_p.tile([128, HCOLS], i16, name=f"hi_{st}_sb")
                lo_sb[st] = p_p.tile([128, S], f32, name=f"lo_{st}_sb")
                nc.sync.dma_start(hi_sb[st][:], his[st][:])
                nc.sync.dma_start(lo_sb[st][:], los[st][:])
            iota_sb = p_p.tile([128, 64], f32)
            nc.sync.dma_start(iota_sb[:], iota[:])

            # ---- rel rowsums -> Rg (natural order, zero tail) ----
            r_in = p_p.tile([128, 7, 128], f32)
            r_in2 = p_p.tile([104, 128], f32)
            nc.sync.dma_start(r_in[:], rel[0:896, :].rearrange("(n p) d -> p n d", p=128))
            nc.sync.dma_start(r_in2[:], rel[896:1000, :])
            rs = p_p.tile([128, 32], f32)
            nc.vector.memset(rs[:], 0.0)
            nc.vector.tensor_reduce(rs[:, 0:7], r_in[:], axis=AX.X, op=OP.add)
            nc.vector.tensor_reduce(rs[:104, 7:8], r_in2[:], axis=AX.X, op=OP.add)
            rtr = p_p.tile([128, 32], f32)
            nc.vector.transpose(rtr[:], rs[:])
            rg_v = rg[:].rearrange("(t a j) -> a t j", t=8, a=4, j=32)
            for a in range(4):
                nc.sync.dma_start(rg_v[a], rtr[32 * a:32 * a + 8, :])

            # ---- phase 1: stream ent shard, rowsum ----
            s_sb = p_p.tile([128, TILES], f32)
            entv = ent[:].rearrange("(n p) d -> p n d", p=128)
            for b in range(NB):
                st_t = p_st.tile([128, BATCH, 128], f32, tag="st", name="st_t")
                nc.sync.dma_start(st_t[:], entv[:, BATCH * b:BATCH * (b + 1), :])
                nc.vector.tensor_reduce(
                    s_sb[:, BATCH * b:BATCH * (b + 1)], st_t[:], axis=AX.X, op=OP.add)

            # ---- phase 1b: transpose to natural order, all-gather ----
            tr = p_p.tile([128, TILES], f32)
            for g in range(NG):
                nc.vector.transpose(tr[:, 32 * g:32 * g + 32], s_sb[:, 32 * g:32 * g + 32])
            agv = ag_in[:].rearrange("(g i a j) -> a i g j", g=NG, i=32, a=4, j=32)
            for a in range(4):
                nc.sync.dma_start(
                    agv[a],
                    tr[32 * a:32 * a + 32, :].rearrange("i (g j) -> i g j", g=NG))
            if not cfg.get("skip_collective"):
                nc.gpsimd.collective_compute(
                    "AllGather", OP.bypass,
                    replica_groups=[list(range(N_CORES))],
                    ins=[ag_in[:].opt()], outs=[sg[:].opt()])

            # ---- phase 2: two-level gathers + one-hot select ----
            sgv = sg[:].rearrange("(n e) -> n e", e=64)     # [SG_ROWS, 64]
            rgv = rg[:].rearrange("(n e) -> n e", e=64)     # [16, 64]
            score = p_p.tile([128, S], f32)
            rel_score = p_p.tile([128, S], f32)
            qctr = [0]

            def sel_chunk(st, table_v, k, dst_ap):
                """Gather chunk k of stream st, select lanes, write [128, SC]."""
                hi_slice = hi_sb[st][:, CHW * k:CHW * (k + 1)]
                gout = p_go.tile([128, SC, 64], f32, tag="gout", name="gout")
                if not cfg.get("skip_gather"):
                    nc.gpsimd.dma_gather(
                        gout[:], table_v, hi_slice, JCH, JCH, 64,
                        single_packet=False, queue_num=qctr[0] % NQ)
                qctr[0] += 1
                mk = p_mk.tile([128, SC, 64], f32, tag="mk", name="mk")
                lo_b = (lo_sb[st][:, SC * k:SC * (k + 1)]
                        .rearrange("p (s o) -> p s o", o=1).to_broadcast([128, SC, 64]))
                io_b = (iota_sb[:].rearrange("p (o e) -> p o e", o=1)
                        .to_broadcast([128, SC, 64]))
                nc.vector.tensor_tensor(mk[:], io_b, lo_b, op=OP.is_equal)
                nc.vector.tensor_tensor(mk[:], mk[:], gout[:], op=OP.mult)
                nc.vector.tensor_reduce(dst_ap, mk[:], axis=AX.X, op=OP.add)

            # rel stream first: overlaps phase 1 (depends only on Rg)
            for k in range(NCH):
                sel_chunk("rel", rgv, k, rel_score[:, SC * k:SC * (k + 1)])
            for k in range(NCH):
                ssl = slice(SC * k, SC * (k + 1))
                sel_chunk("src", sgv, k, score[:, ssl])
                d_t = p_mk.tile([128, SC], f32, tag="dsel", name="d_t")
                sel_chunk("dst", sgv, k, d_t[:])
                nc.vector.tensor_tensor(score[:, ssl], score[:, ssl], d_t[:],
                                        op=OP.subtract)
                nc.vector.tensor_tensor(score[:, ssl], score[:, ssl],
                                        rel_score[:, ssl], op=OP.add)

            # ---- phase 3: scores -> out (natural edge order) ----
            tr2 = p_p.tile([128, S], f32)
            for g in range(NG2):
                nc.vector.transpose(tr2[:, 32 * g:32 * g + 32],
                                    score[:, 32 * g:32 * g + 32])
            ov = out[0:NEDGE].rearrange("(g i a j) -> a i g j", g=NG2, i=32, a=4, j=32)
            for a in range(4):
                nc.sync.dma_start(
                    ov[a],
                    tr2[32 * a:32 * a + 32, :].rearrange("i (g j) -> i g j", g=NG2))

            z = p_p.tile([16, PAD // 16], f32)
            nc.vector.memset(z[:], 0.0)
            nc.sync.dma_start(out[NEDGE:SCORE].rearrange("(p f) -> p f", p=16), z[:])

    nc.finalize()
    return nc


_NC_CACHE = {}


def _get_nc(key, cfg):
    if key not in _NC_CACHE:
        _NC_CACHE[key] = build_nc(cfg)
    return _NC_CACHE[key]


def _prep_idx(raw):
    """raw int array [NEDGE] -> (hi wrapped+replicated int16, lo f32)."""
    raw = np.asarray(raw).astype(np.int64)
    nedge = raw.shape[0]
    hi = (raw >> 6).astype(np.int16)
    lo = (raw & 63).astype(np.float32)
    hi_w = np.tile(np.ascontiguousarray(hi.reshape(nedge // 16, 16).T), (8, 1))
    lo_t = np.ascontiguousarray(lo.reshape(nedge // 128, 128).T)
    return np.ascontiguousarray(hi_w), lo_t


def shard_inputs(ent_table, rel_table, src_idx, dst_idx, rel_idx, cfg):
    ROWS = cfg["tiles"] * 128
    n_ent = np.asarray(ent_table).shape[0]
    ent = np.ascontiguousarray(np.asarray(ent_table, dtype=np.float32))
    relt = np.ascontiguousarray(np.asarray(rel_table, dtype=np.float32))
    idxs = {"src": np.asarray(src_idx), "dst": np.asarray(dst_idx),
            "rel": np.asarray(rel_idx)}
    in_maps = []
    for c in range(N_CORES):
        lo_r = c * ROWS
        hi_r = min((c + 1) * ROWS, n_ent)
        shard = ent[lo_r:hi_r]
        if hi_r - lo_r < ROWS:
            pad = np.zeros((ROWS - max(hi_r - lo_r, 0), DIM), np.float32)
            shard = np.concatenate([shard, pad], axis=0) if hi_r > lo_r else pad
        m = {"ent_shard": shard, "rel_table": relt}
        for st in ("src", "dst", "rel"):
            hi_w, lo_t = _prep_idx(idxs[st][c])
            m[f"hi_{st}"] = hi_w
            m[f"lo_{st}"] = lo_t
        in_maps.append(m)
    return in_maps


def kernel(ent_table, rel_table, src_idx, dst_idx, rel_idx):
    from concourse.bass_utils import run_bass_kernel_spmd

    cfg = FULL_CFG
    nc = _get_nc("full", cfg)
    in_maps = shard_inputs(ent_table, rel_table, src_idx, dst_idx, rel_idx, cfg)
    res = run_bass_kernel_spmd(nc, in_maps, core_ids=list(range(N_CORES)))
    return np.concatenate([res.results[c]["out"] for c in range(N_CORES)])


# revision 5
# speedup vs baseline: 1.3102x; 1.3102x over previous
"""TransE-style GNN message-passing scoring kernel for 8 Trainium2 NeuronCores.

Math: reference computes scores[r,e] = sum_d(ent[src]+rel[rl]-ent[dst])[d].
The sum over d is linear, so scores = S[src] + R[rl] - S[dst] where
S = rowsum(ent_table) [1M], R = rowsum(rel_table) [1000].

Per-core plan (SPMD, core c handles relation row c):
  phase 1: stream 1/8 of ent_table (992 tiles of 128 rows), DVE-reduce each
           [128,128] tile over the free axis -> S-chunk in SBUF [128, 992].
  phase 1b: 32x32 stream-transposes + block-permuting DMA write the chunk to
           DRAM in natural row order; AllGather -> Sg[1,015,808] f32 = S.
           rel rowsums computed locally -> Rg[1024] natural order.
  phase 2: two-level gather: hi = idx>>6 (int16, host-prepped in the SWDGE
           wrapped layout) drives dma_gather of 64-f32 granules from
           Sg viewed [15872, 64] across 4 SWDGE queues; the within-granule
           element is selected on DVE with an iota/is_equal one-hot against
           lo = idx&63 (host-prepped f32), multiply + reduce.
  phase 3: score = sel(src) + sel(rel) - sel(dst); stream-transpose +
           block-permuted DMA writes out[131072] in edge order; zero tail.
Host does only integer index prep (hi/lo split + SWDGE wrap layout) and
tensor sharding/concat; all FP math runs on device.
"""

import numpy as np

N_ENT = 1_000_000
DIM = 128
R_TYPES = 8
E_PER_TYPE = 131_072
SCORE_DIM = 150_000
N_REL = 1_000
N_CORES = 8

FULL_CFG = dict(
    tiles=992,          # ent tiles of 128 rows per core (992*128 = 126,976)
    e_cols=E_PER_TYPE // 128,   # 1024 score slots ([128, e_cols] per core)
    jch=2048,           # indices per dma_gather instruction
    score_dim=SCORE_DIM,
    batch=16,           # stream tiles per DMA batch
    queues=4,
)


def build_nc(cfg):
    import concourse.bass as bass
    import concourse.bacc as bacc
    import concourse.tile as tile
    from concourse import mybir

    f32 = mybir.dt.float32
    i16 = mybir.dt.int16
    AX = mybir.AxisListType
    OP = mybir.AluOpType

    TILES = cfg["tiles"]
    S = cfg["e_cols"]              # score slots (free dim of [128, S])
    JCH = cfg["jch"]
    SCORE = cfg["score_dim"]
    BATCH = cfg["batch"]
    NQ = cfg["queues"]
    ROWS = TILES * 128
    SG_LEN = N_CORES * ROWS        # all-gathered S length (>= N_ENT)
    SG_ROWS = SG_LEN // 64
    NEDGE = 128 * S
    HCOLS = NEDGE // 16            # wrapped-idx columns
    NB = TILES // BATCH
    NG = TILES // 32
    NCH = NEDGE // JCH             # gather chunks per stream
    SC = JCH // 128                # score slots per chunk
    CHW = JCH // 16                # hi columns per chunk
    NG2 = S // 32
    assert TILES % BATCH == 0 and TILES % 32 == 0 and NEDGE % JCH == 0
    assert JCH % 128 == 0 and S % 32 == 0
    PAD = SCORE - NEDGE
    assert PAD % 16 == 0

    nc = bacc.Bacc(None, num_devices=N_CORES, num_swdge_queues=NQ)
    ent = nc.dram_tensor("ent_shard", [ROWS, DIM], f32, kind="ExternalInput")
    rel = nc.dram_tensor("rel_table", [N_REL, DIM], f32, kind="ExternalInput")
    his = {}
    los = {}
    for st in ("src", "dst", "rel"):
        his[st] = nc.dram_tensor(f"hi_{st}", [128, HCOLS], i16, kind="ExternalInput")
        los[st] = nc.dram_tensor(f"lo_{st}", [128, S], f32, kind="ExternalInput")
    out = nc.dram_tensor("out", [SCORE], f32, kind="ExternalOutput")
    iota = nc.inline_tensor(
        np.tile(np.arange(64, dtype=np.float32), (128, 1)), name="iota64")

    with tile.TileContext(nc) as tc:
        with tc.tile_pool(name="stream", bufs=3) as p_st, \
             tc.tile_pool(name="persist", bufs=1) as p_p, \
             tc.tile_pool(name="gout", bufs=6) as p_go, \
             tc.tile_pool(name="mask", bufs=4) as p_mk, \
             tc.tile_pool(name="dram", bufs=1, space="DRAM") as p_d:

            ag_in = p_d.tile([ROWS], f32)
            sg = p_d.tile([SG_LEN], f32, addr_space="Shared")
            rg = p_d.tile([1024], f32)

            # ---- index metadata loads (early) ----
            hi_sb = {}
            lo_sb = {}
            for st in ("src", "dst", "rel"):
                hi_sb[st] = p_p.tile([128, HCOLS], i16, name=f"hi_{st}_sb")
                lo_sb[st] = p_p.tile([128, S], f32, name=f"lo_{st}_sb")
                nc.sync.dma_start(hi_sb[st][:], his[st][:])
                nc.sync.dma_start(lo_sb[st][:], los[st][:])
            iota_sb = p_p.tile([128, 64], f32)
            nc.sync.dma_start(iota_sb[:], iota[:])

            # ---- rel rowsums -> Rg (natural order, zero tail) ----
            r_in = p_p.tile([128, 7, 128], f32)
            r_in2 = p_p.tile([104, 128], f32)
            nc.sync.dma_start(r_in[:], rel[0:896, :].rearrange("(n p) d -> p n d", p=128))
            nc.sync.dma_start(r_in2[:], rel[896:1000, :])
            rs = p_p.tile([128, 32], f32)
            nc.vector.memset(rs[:], 0.0)
            nc.vector.tensor_reduce(rs[:, 0:7], r_in[:], axis=AX.X, op=OP.add)
            nc.vector.tensor_reduce(rs[:104, 7:8], r_in2[:], axis=AX.X, op=OP.add)
            rtr = p_p.tile([128, 32], f32)
            nc.vector.transpose(rtr[:], rs[:])
            rg_v = rg[:].rearrange("(t a j) -> a t j", t=8, a=4, j=32)
            for a in range(4):
                nc.sync.dma_start(rg_v[a], rtr[32 * a:32 * a + 8, :])

            # ---- phase 1: stream ent shard, rowsum ----
            s_sb = p_p.tile([128, TILES], f32)
            entv = ent[:].rearrange("(n p) d -> p n d", p=128)
            for b in range(NB):
                st_t = p_st.tile([128, BATCH, 128], f32, tag="st", name="st_t")
                nc.sync.dma_start(st_t[:], entv[:, BATCH * b:BATCH * (b + 1), :])
                nc.vector.tensor_reduce(
                    s_sb[:, BATCH * b:BATCH * (b + 1)], st_t[:], axis=AX.X, op=OP.add)

            # ---- phase 1b: transpose to natural order, all-gather ----
            tr = p_p.tile([128, TILES], f32)
            for g in range(NG):
                nc.vector.transpose(tr[:, 32 * g:32 * g + 32], s_sb[:, 32 * g:32 * g + 32])
            agv = ag_in[:].rearrange("(g i a j) -> a i g j", g=NG, i=32, a=4, j=32)
            for a in range(4):
                nc.sync.dma_start(
                    agv[a],
                    tr[32 * a:32 * a + 32, :].rearrange("i (g j) -> i g j", g=NG))
            if not cfg.get("skip_collective"):
                nc.gpsimd.collective_compute(
                    "AllGather", OP.bypass,
                    replica_groups=[list(range(N_CORES))],
                    ins=[ag_in[:].opt()], outs=[sg[:].opt()])

            # ---- phase 2: two-level gathers + one-hot select ----
            sgv = sg[:].rearrange("(n e) -> n e", e=64)     # [SG_ROWS, 64]
            rgv = rg[:].rearrange("(n e) -> n e", e=64)     # [16, 64]
            score = p_p.tile([128, S], f32)
            rel_score = p_p.tile([128, S], f32)
            qctr = [0]

            def sel_chunk(st, table_v, k, dst_ap):
                """Gather chunk k of stream st, select lanes, write [128, SC]."""
                hi_slice = hi_sb[st][:, CHW * k:CHW * (k + 1)]
                gout = p_go.tile([128, SC, 64], f32, tag="gout", name="gout")
                if not cfg.get("skip_gather"):
                    nc.gpsimd.dma_gather(
                        gout[:], table_v, hi_slice, JCH, JCH, 64,
                        single_packet=False, queue_num=qctr[0] % NQ)
                qctr[0] += 1
                mk = p_mk.tile([128, SC, 64], f32, tag="mk", name="mk")
                lo_b = (lo_sb[st][:, SC * k:SC * (k + 1)]
                        .rearrange("p (s o) -> p s o", o=1).to_broadcast([128, SC, 64]))
                io_b = (iota_sb[:].rearrange("p (o e) -> p o e", o=1)
                        .to_broadcast([128, SC, 64]))
                nc.vector.tensor_tensor(mk[:], io_b, lo_b, op=OP.is_equal)
                nc.vector.tensor_tensor(mk[:], mk[:], gout[:], op=OP.mult)
                nc.vector.tensor_reduce(dst_ap, mk[:], axis=AX.X, op=OP.add)

            # Emit enough rel chunks first to keep Pool DGE busy during
            # phase 1 (rel gathers depend only on Rg, not the AllGather);
            # interleave the rest so Pool never idles.
            NPRE = min(cfg.get("rel_pre", NCH // 2), NCH)
            for k in range(NPRE):
                sel_chunk("rel", rgv, k, rel_score[:, SC * k:SC * (k + 1)])
            for k in range(NCH):
                ssl = slice(SC * k, SC * (k + 1))
                sel_chunk("src", sgv, k, score[:, ssl])
                d_t = p_mk.tile([128, SC], f32, tag="dsel", name="d_t")
                sel_chunk("dst", sgv, k, d_t[:])
                if NPRE + k < NCH:
                    kk = NPRE + k
                    sel_chunk("rel", rgv, kk, rel_score[:, SC * kk:SC * (kk + 1)])
                nc.vector.tensor_tensor(score[:, ssl], score[:, ssl], d_t[:],
                                        op=OP.subtract)
                nc.vector.tensor_tensor(score[:, ssl], score[:, ssl],
                                        rel_score[:, ssl], op=OP.add)

            # ---- phase 3: scores -> out (natural edge order) ----
            tr2 = p_p.tile([128, S], f32)
            for g in range(NG2):
                nc.vector.transpose(tr2[:, 32 * g:32 * g + 32],
                                    score[:, 32 * g:32 * g + 32])
            ov = out[0:NEDGE].rearrange("(g i a j) -> a i g j", g=NG2, i=32, a=4, j=32)
            for a in range(4):
                nc.sync.dma_start(
                    ov[a],
                    tr2[32 * a:32 * a + 32, :].rearrange("i (g j) -> i g j", g=NG2))

            z = p_p.tile([16, PAD // 16], f32)
            nc.vector.memset(z[:], 0.0)
            nc.sync.dma_start(out[NEDGE:SCORE].rearrange("(p f) -> p f", p=16), z[:])

    nc.finalize()
    return nc


_NC_CACHE = {}


def _get_nc(key, cfg):
    if key not in _NC_CACHE:
        _NC_CACHE[key] = build_nc(cfg)
    return _NC_CACHE[key]


def _prep_idx(raw):
    """raw int array [NEDGE] -> (hi wrapped+replicated int16, lo f32)."""
    raw = np.asarray(raw).astype(np.int64)
    nedge = raw.shape[0]
    hi = (raw >> 6).astype(np.int16)
    lo = (raw & 63).astype(np.float32)
    hi_w = np.tile(np.ascontiguousarray(hi.reshape(nedge // 16, 16).T), (8, 1))
    lo_t = np.ascontiguousarray(lo.reshape(nedge // 128, 128).T)
    return np.ascontiguousarray(hi_w), lo_t


def shard_inputs(ent_table, rel_table, src_idx, dst_idx, rel_idx, cfg):
    ROWS = cfg["tiles"] * 128
    n_ent = np.asarray(ent_table).shape[0]
    ent = np.ascontiguousarray(np.asarray(ent_table, dtype=np.float32))
    relt = np.ascontiguousarray(np.asarray(rel_table, dtype=np.float32))
    idxs = {"src": np.asarray(src_idx), "dst": np.asarray(dst_idx),
            "rel": np.asarray(rel_idx)}
    in_maps = []
    for c in range(N_CORES):
        lo_r = c * ROWS
        hi_r = min((c + 1) * ROWS, n_ent)
        shard = ent[lo_r:hi_r]
        if hi_r - lo_r < ROWS:
            pad = np.zeros((ROWS - max(hi_r - lo_r, 0), DIM), np.float32)
            shard = np.concatenate([shard, pad], axis=0) if hi_r > lo_r else pad
        m = {"ent_shard": shard, "rel_table": relt}
        for st in ("src", "dst", "rel"):
            hi_w, lo_t = _prep_idx(idxs[st][c])
            m[f"hi_{st}"] = hi_w
            m[f"lo_{st}"] = lo_t
        in_maps.append(m)
    return in_maps


def kernel(ent_table, rel_table, src_idx, dst_idx, rel_idx):
    from concourse.bass_utils import run_bass_kernel_spmd

    cfg = FULL_CFG
    nc = _get_nc("full", cfg)
    in_maps = shard_inputs(ent_table, rel_table, src_idx, dst_idx, rel_idx, cfg)
    res = run_bass_kernel_spmd(nc, in_maps, core_ids=list(range(N_CORES)))
    return np.concatenate([res.results[c]["out"] for c in range(N_CORES)])


# revision 6
# speedup vs baseline: 1.4171x; 1.0815x over previous
"""TransE-style GNN message-passing scoring kernel for 8 Trainium2 NeuronCores.

Math: reference computes scores[r,e] = sum_d(ent[src]+rel[rl]-ent[dst])[d].
The sum over d is linear, so scores = S[src] + R[rl] - S[dst] where
S = rowsum(ent_table) [1M], R = rowsum(rel_table) [1000].

Per-core plan (SPMD, core c handles relation row c):
  phase 1: stream 1/8 of ent_table (992 tiles of 128 rows), DVE-reduce each
           [128,128] tile over the free axis -> S-chunk in SBUF [128, 992].
  phase 1b: 32x32 stream-transposes + block-permuting DMA write the chunk to
           DRAM in natural row order; AllGather -> Sg[1,015,808] f32 = S.
           rel rowsums computed locally -> Rg[1024] natural order.
  phase 2: two-level gather: hi = idx>>6 (int16, host-prepped in the SWDGE
           wrapped layout) drives dma_gather of 64-f32 granules from
           Sg viewed [15872, 64] across 4 SWDGE queues; the within-granule
           element is selected on DVE with an iota/is_equal one-hot against
           lo = idx&63 (host-prepped f32), multiply + reduce.
  phase 3: score = sel(src) + sel(rel) - sel(dst); stream-transpose +
           block-permuted DMA writes out[131072] in edge order; zero tail.
Host does only integer index prep (hi/lo split + SWDGE wrap layout) and
tensor sharding/concat; all FP math runs on device.
"""

import numpy as np

N_ENT = 1_000_000
DIM = 128
R_TYPES = 8
E_PER_TYPE = 131_072
SCORE_DIM = 150_000
N_REL = 1_000
N_CORES = 8

FULL_CFG = dict(
    tiles=992,          # ent tiles of 128 rows per core (992*128 = 126,976)
    e_cols=E_PER_TYPE // 128,   # 1024 score slots ([128, e_cols] per core)
    jch=2048,           # indices per dma_gather instruction
    score_dim=SCORE_DIM,
    batch=16,           # stream tiles per DMA batch
    queues=4,
)


def build_nc(cfg):
    import concourse.bass as bass
    import concourse.bacc as bacc
    import concourse.tile as tile
    from concourse import mybir

    f32 = mybir.dt.float32
    i16 = mybir.dt.int16
    AX = mybir.AxisListType
    OP = mybir.AluOpType

    TILES = cfg["tiles"]
    S = cfg["e_cols"]              # score slots (free dim of [128, S])
    JCH = cfg["jch"]
    SCORE = cfg["score_dim"]
    BATCH = cfg["batch"]
    NQ = cfg["queues"]
    ROWS = TILES * 128
    SG_LEN = N_CORES * ROWS        # all-gathered S length (>= N_ENT)
    SG_ROWS = SG_LEN // 64
    NEDGE = 128 * S
    HCOLS = NEDGE // 16            # wrapped-idx columns
    NB = TILES // BATCH
    NG = TILES // 32
    NCH = NEDGE // JCH             # gather chunks per stream
    SC = JCH // 128                # score slots per chunk
    CHW = JCH // 16                # hi columns per chunk
    NG2 = S // 32
    assert TILES % BATCH == 0 and TILES % 32 == 0 and NEDGE % JCH == 0
    assert JCH % 128 == 0 and S % 32 == 0
    PAD = SCORE - NEDGE
    assert PAD % 16 == 0

    nc = bacc.Bacc(None, num_devices=N_CORES, num_swdge_queues=NQ)
    ent = nc.dram_tensor("ent_shard", [ROWS, DIM], f32, kind="ExternalInput")
    rel = nc.dram_tensor("rel_table", [N_REL, DIM], f32, kind="ExternalInput")
    his = {}
    los = {}
    for st in ("src", "dst", "rel"):
        his[st] = nc.dram_tensor(f"hi_{st}", [128, HCOLS], i16, kind="ExternalInput")
        los[st] = nc.dram_tensor(f"lo_{st}", [128, S], f32, kind="ExternalInput")
    out = nc.dram_tensor("out", [SCORE], f32, kind="ExternalOutput")
    iota = nc.inline_tensor(
        np.tile(np.arange(64, dtype=np.float32), (128, 1)), name="iota64")

    with tile.TileContext(nc) as tc:
        with tc.tile_pool(name="stream", bufs=3) as p_st, \
             tc.tile_pool(name="persist", bufs=1) as p_p, \
             tc.tile_pool(name="gout", bufs=6) as p_go, \
             tc.tile_pool(name="mask", bufs=4) as p_mk, \
             tc.tile_pool(name="dram", bufs=1, space="DRAM") as p_d:

            ag_in = p_d.tile([ROWS], f32)
            sg = p_d.tile([SG_LEN], f32, addr_space="Shared")
            rg = p_d.tile([1024], f32)

            # ---- index metadata loads (early) ----
            hi_sb = {}
            lo_sb = {}
            for st in ("src", "dst", "rel"):
                hi_sb[st] = p_p.tile([128, HCOLS], i16, name=f"hi_{st}_sb")
                lo_sb[st] = p_p.tile([128, S], f32, name=f"lo_{st}_sb")
                nc.sync.dma_start(hi_sb[st][:], his[st][:])
                nc.sync.dma_start(lo_sb[st][:], los[st][:])
            iota_sb = p_p.tile([128, 64], f32)
            nc.sync.dma_start(iota_sb[:], iota[:])

            # ---- rel rowsums -> Rg (natural order via super-row layout) ----
            # partition p holds rel rows [8p, 8p+8); 125*8 = 1000 exactly.
            r_in = p_p.tile([125, 8, 128], f32)
            nc.sync.dma_start(r_in[:], rel[0:1000, :].rearrange("(p b) d -> p b d", b=8))
            rs = p_p.tile([128, 8], f32)
            nc.vector.memset(rs[:], 0.0)
            nc.vector.tensor_reduce(rs[:125, :], r_in[:], axis=AX.X, op=OP.add)
            nc.sync.dma_start(rg[:].rearrange("(p b) -> p b", b=8), rs[:])

            # ---- phase 1: stream ent shard (super-row: partition p reads
            # BATCH consecutive rows => 8KB-contiguous DMA descriptors),
            # rowsum lands directly in natural-order-compatible layout:
            # s_sb[p, BATCH*j + b] = S[base + j*128*BATCH + p*BATCH + b]
            s_sb = p_p.tile([128, TILES], f32)
            for j in range(NB):
                r0 = j * 128 * BATCH
                st_t = p_st.tile([128, BATCH, 128], f32, tag="st", name="st_t")
                nc.sync.dma_start(
                    st_t[:],
                    ent[r0:r0 + 128 * BATCH, :].rearrange("(p b) d -> p b d", b=BATCH))
                nc.vector.tensor_reduce(
                    s_sb[:, BATCH * j:BATCH * (j + 1)], st_t[:], axis=AX.X, op=OP.add)

            # ---- phase 1b: single natural-order write + all-gather ----
            nc.sync.dma_start(
                ag_in[:].rearrange("(j p b) -> p j b", j=NB, p=128, b=BATCH),
                s_sb[:].rearrange("p (j b) -> p j b", b=BATCH))
            if not cfg.get("skip_collective"):
                nc.gpsimd.collective_compute(
                    "AllGather", OP.bypass,
                    replica_groups=[list(range(N_CORES))],
                    ins=[ag_in[:].opt()], outs=[sg[:].opt()])

            # ---- phase 2: two-level gathers + one-hot select ----
            sgv = sg[:].rearrange("(n e) -> n e", e=64)     # [SG_ROWS, 64]
            rgv = rg[:].rearrange("(n e) -> n e", e=64)     # [16, 64]
            score = p_p.tile([128, S], f32)
            rel_score = p_p.tile([128, S], f32)
            qctr = [0]

            def sel_chunk(st, table_v, k, dst_ap):
                """Gather chunk k of stream st, select lanes, write [128, SC]."""
                hi_slice = hi_sb[st][:, CHW * k:CHW * (k + 1)]
                gout = p_go.tile([128, SC, 64], f32, tag="gout", name="gout")
                if not cfg.get("skip_gather"):
                    nc.gpsimd.dma_gather(
                        gout[:], table_v, hi_slice, JCH, JCH, 64,
                        single_packet=False, queue_num=qctr[0] % NQ)
                qctr[0] += 1
                mk = p_mk.tile([128, SC, 64], f32, tag="mk", name="mk")
                lo_b = (lo_sb[st][:, SC * k:SC * (k + 1)]
                        .rearrange("p (s o) -> p s o", o=1).to_broadcast([128, SC, 64]))
                io_b = (iota_sb[:].rearrange("p (o e) -> p o e", o=1)
                        .to_broadcast([128, SC, 64]))
                nc.vector.tensor_tensor(mk[:], io_b, lo_b, op=OP.is_equal)
                nc.vector.tensor_tensor(mk[:], mk[:], gout[:], op=OP.mult)
                nc.vector.tensor_reduce(dst_ap, mk[:], axis=AX.X, op=OP.add)

            # Emit enough rel chunks first to keep Pool DGE busy during
            # phase 1 (rel gathers depend only on Rg, not the AllGather);
            # interleave the rest so Pool never idles.
            NPRE = min(cfg.get("rel_pre", 34), NCH)
            for k in range(NPRE):
                sel_chunk("rel", rgv, k, rel_score[:, SC * k:SC * (k + 1)])
            for k in range(NCH):
                ssl = slice(SC * k, SC * (k + 1))
                sel_chunk("src", sgv, k, score[:, ssl])
                d_t = p_mk.tile([128, SC], f32, tag="dsel", name="d_t")
                sel_chunk("dst", sgv, k, d_t[:])
                if NPRE + k < NCH:
                    kk = NPRE + k
                    sel_chunk("rel", rgv, kk, rel_score[:, SC * kk:SC * (kk + 1)])
                nc.vector.tensor_tensor(score[:, ssl], score[:, ssl], d_t[:],
                                        op=OP.subtract)
                nc.vector.tensor_tensor(score[:, ssl], score[:, ssl],
                                        rel_score[:, ssl], op=OP.add)

            # ---- phase 3: scores -> out (natural edge order) ----
            tr2 = p_p.tile([128, S], f32)
            for g in range(NG2):
                nc.vector.transpose(tr2[:, 32 * g:32 * g + 32],
                                    score[:, 32 * g:32 * g + 32])
            ov = out[0:NEDGE].rearrange("(g i a j) -> a i g j", g=NG2, i=32, a=4, j=32)
            for a in range(4):
                nc.sync.dma_start(
                    ov[a],
                    tr2[32 * a:32 * a + 32, :].rearrange("i (g j) -> i g j", g=NG2))

            z = p_p.tile([16, PAD // 16], f32)
            nc.vector.memset(z[:], 0.0)
            nc.sync.dma_start(out[NEDGE:SCORE].rearrange("(p f) -> p f", p=16), z[:])

    nc.finalize()
    return nc


_NC_CACHE = {}


def _get_nc(key, cfg):
    if key not in _NC_CACHE:
        _NC_CACHE[key] = build_nc(cfg)
    return _NC_CACHE[key]


def _prep_idx(raw):
    """raw int array [NEDGE] -> (hi wrapped+replicated int16, lo f32)."""
    raw = np.asarray(raw).astype(np.int64)
    nedge = raw.shape[0]
    hi = (raw >> 6).astype(np.int16)
    lo = (raw & 63).astype(np.float32)
    hi_w = np.tile(np.ascontiguousarray(hi.reshape(nedge // 16, 16).T), (8, 1))
    lo_t = np.ascontiguousarray(lo.reshape(nedge // 128, 128).T)
    return np.ascontiguousarray(hi_w), lo_t


def shard_inputs(ent_table, rel_table, src_idx, dst_idx, rel_idx, cfg):
    ROWS = cfg["tiles"] * 128
    n_ent = np.asarray(ent_table).shape[0]
    ent = np.ascontiguousarray(np.asarray(ent_table, dtype=np.float32))
    relt = np.ascontiguousarray(np.asarray(rel_table, dtype=np.float32))
    idxs = {"src": np.asarray(src_idx), "dst": np.asarray(dst_idx),
            "rel": np.asarray(rel_idx)}
    in_maps = []
    for c in range(N_CORES):
        lo_r = c * ROWS
        hi_r = min((c + 1) * ROWS, n_ent)
        shard = ent[lo_r:hi_r]
        if hi_r - lo_r < ROWS:
            pad = np.zeros((ROWS - max(hi_r - lo_r, 0), DIM), np.float32)
            shard = np.concatenate([shard, pad], axis=0) if hi_r > lo_r else pad
        m = {"ent_shard": shard, "rel_table": relt}
        for st in ("src", "dst", "rel"):
            hi_w, lo_t = _prep_idx(idxs[st][c])
            m[f"hi_{st}"] = hi_w
            m[f"lo_{st}"] = lo_t
        in_maps.append(m)
    return in_maps


def kernel(ent_table, rel_table, src_idx, dst_idx, rel_idx):
    from concourse.bass_utils import run_bass_kernel_spmd

    cfg = FULL_CFG
    nc = _get_nc("full", cfg)
    in_maps = shard_inputs(ent_table, rel_table, src_idx, dst_idx, rel_idx, cfg)
    res = run_bass_kernel_spmd(nc, in_maps, core_ids=list(range(N_CORES)))
    return np.concatenate([res.results[c]["out"] for c in range(N_CORES)])
